# revision 23
# baseline (speedup 1.0000x reference)
# Trainium2 Bass kernel for nn_ExtendedSpatialAttention.
#
# Sharding: 16 (clip, frame) rows across 8 cores -> 2 frames per core
# (core c: clip b=c//4, frames 2j, 2j+1, j=c%4). Each core receives its two
# frames plus the 1-frame halo (frame 2j-1; frame 0 duplicated for j=0 --
# attention over a duplicated key set equals the single-frame window). No
# inter-core communication is needed.
#
# Device dataflow is feature-major ([C, tokens]); attention uses the
# "S-transposed" layout (keys on partitions): softmax denominators come from
# a ones-augmented V column in the PV matmul, so no transposes and no
# cross-partition reductions appear anywhere. LayerNorm affines are folded
# into projection weights on the host; softmax skips max-subtraction (scores
# are O(1), and the reference's global-max shift cancels mathematically).
import sys
import numpy as np

sys.path.insert(0, "/opt/trn_rl_repo")

import ml_dtypes

BF16 = ml_dtypes.bfloat16
F32 = np.float32
EPS = 1e-5
N_CORES = 8
C = 512
CH = 4            # channel chunks of 128
NH = 8            # heads
HD = 64           # head dim
T = 8             # frames per clip
B = 2             # clips
NT = 77           # text tokens


def build_module(HW=1024, KCG=2, PHASES=99, ATTP=99):
    import contextlib
    import concourse.bacc as bacc
    import concourse.mybir as mybir
    import concourse.tile as tile

    f32, bf = mybir.dt.float32, mybir.dt.bfloat16
    OP = mybir.AluOpType
    AF = mybir.ActivationFunctionType
    AX = mybir.AxisListType

    NTC = HW // 128
    NKC = 2 * NTC
    SOFF = max(HW, 512)

    nc = bacc.Bacc("TRN2", target_bir_lowering=False, debug=False,
                   enable_asserts=False, num_devices=N_CORES)

    xin = nc.dram_tensor("xin", [3, CH, 128, HW], f32, kind="ExternalInput").ap()
    ctxin = nc.dram_tensor("ctxin", [2, CH, 128, NT], f32, kind="ExternalInput").ap()
    outD = nc.dram_tensor("out", [2, CH, 128, HW], f32, kind="ExternalOutput").ap()
    gnwD = nc.dram_tensor("gnw", [2, CH, 128, 1], f32, kind="ExternalInput").ap()
    gnbD = nc.dram_tensor("gnb", [2, CH, 128, 1], f32, kind="ExternalInput").ap()
    gsumD = nc.dram_tensor("gsum", [128, 8], f32, kind="ExternalInput").ap()
    e8D = nc.dram_tensor("e8", [8, 128], f32, kind="ExternalInput").ap()
    selD = nc.dram_tensor("sel", [2, 128], f32, kind="ExternalInput").ap()
    biasD = nc.dram_tensor("bias", [8, 512], bf, kind="ExternalInput").ap()
    wD = {}
    for name in ("wq", "wk", "wv", "wo", "cawq", "cawk", "cawv", "cawo"):
        wD[name] = nc.dram_tensor(name, [CH, 128, 512], bf, kind="ExternalInput").ap()
    for name in ("diag", "cadiag"):
        wD[name] = nc.dram_tensor(name, [CH, 128, 128], bf, kind="ExternalInput").ap()

    with tile.TileContext(nc) as tc:
        with contextlib.ExitStack() as st:
            wp = st.enter_context(tc.tile_pool(name="wp", bufs=1))
            sp = st.enter_context(tc.tile_pool(name="spool", bufs=1))
            pp = st.enter_context(tc.tile_pool(name="ppool", bufs=1, space="PSUM"))

            BUFS = {
                "xin": 4, "sq": 2, "ss": 6, "nmr": 4, "xnb": 8, "fp": 4,
                "xhat": 7, "xh2": 4, "kT": 8, "vtok": 16, "vctx": 2, "q": 4,
                "expS": 2, "onorm": 4, "rbs": 1, "row1": 4, "ctxr": 6, "ctxin": 5,
                "ctxh": 8, "kctx": 8,
            }
            PBUFS = {"op": 2, "sp": 1}

            uid = [0]

            def nm(p):
                uid[0] += 1
                return f"{p}_{uid[0]}"

            def stile(shape, dtype, tag):
                return sp.tile(shape, dtype, name=nm(tag), tag=tag, bufs=BUFS[tag])

            def ptile(shape, tag):
                return pp.tile(shape, f32, name=nm(tag), tag=tag, bufs=PBUFS[tag])

            # ---------------- constants & weights ----------------
            W = {}
            for name in ("wq", "wk", "wv", "wo", "cawq", "cawk", "cawv", "cawo"):
                W[name] = []
                for c in range(CH):
                    t = wp.tile([128, 512], bf, name=f"{name}{c}")
                    nc.sync.dma_start(out=t[:], in_=wD[name][c])
                    W[name].append(t)
            for name in ("diag", "cadiag"):
                W[name] = []
                for c in range(CH):
                    t = wp.tile([128, 128], bf, name=f"{name}{c}")
                    nc.sync.dma_start(out=t[:], in_=wD[name][c])
                    W[name].append(t)
            gw, gb = [], []
            for g in range(2):
                gw.append([])
                gb.append([])
                for c in range(CH):
                    t = wp.tile([128, 1], f32, name=f"gw{g}{c}")
                    nc.sync.dma_start(out=t[:], in_=gnwD[g, c])
                    gw[g].append(t)
                    t2 = wp.tile([128, 1], f32, name=f"gb{g}{c}")
                    nc.sync.dma_start(out=t2[:], in_=gnbD[g, c])
                    gb[g].append(t2)
            gsum_t = wp.tile([128, 8], f32, name="gsum_t")
            nc.sync.dma_start(out=gsum_t[:], in_=gsumD[:])
            e8_t = wp.tile([8, 128], f32, name="e8_t")
            nc.sync.dma_start(out=e8_t[:], in_=e8D[:])
            bias_tiles = []
            for r in range(8):
                bt = wp.tile([1, 512], bf, name=f"bias{r}")
                nc.sync.dma_start(out=bt[:], in_=biasD[r:r + 1, :])
                bias_tiles.append(bt)
            ones_col = wp.tile([128, 1], f32, name="ones_col")
            nc.vector.memset(ones_col[:], 1.0)
            ones_colb = wp.tile([128, 1], bf, name="ones_colb")
            nc.vector.memset(ones_colb[:], 1.0)
            ones_r1 = wp.tile([1, 128], f32, name="ones_r1")
            nc.vector.memset(ones_r1[:], 1.0)
            ones_bf = wp.tile([1, 512], bf, name="ones_bf")
            nc.vector.memset(ones_bf[:], 1.0)
            eps_t = wp.tile([128, 1], f32, name="eps_t")
            nc.vector.memset(eps_t[:], EPS)

            def halves(nfree):
                return [(h * 512, 512) for h in range(nfree // 512)] or [(0, nfree)]

            # ---------------- GroupNorm + standardize-over-C ----------------
            def norm_block(src, gidx, xn_tag, xn_dtype, nfree, xhat_tag,
                           inplace=False):
                gstats = ptile([8, 8], "op")
                for c in range(CH):
                    sq = stile([128, nfree], f32, "sq")
                    ssum = stile([128, 2], f32, "ss")
                    nc.scalar.activation(out=sq[:], in_=src[c][:], func=AF.Square,
                                         accum_out=ssum[:, 1:2])
                    nc.vector.tensor_reduce(out=ssum[:, 0:1], in_=src[c][:],
                                            axis=AX.X, op=OP.add)
                    nc.tensor.matmul(gstats[0:8, c:c + 1], gsum_t[:, 0:8],
                                     ssum[:, 0:1], start=True, stop=True)
                    nc.tensor.matmul(gstats[0:8, 4 + c:5 + c], gsum_t[:, 0:8],
                                     ssum[:, 1:2], start=True, stop=True)
                gsb = stile([8, 8], f32, "nmr")
                nc.vector.tensor_copy(gsb[:], gstats[:])
                nmr = stile([8, 8], f32, "nmr")
                sc8 = stile([8, 8], f32, "nmr")
                nc.vector.tensor_scalar(out=nmr[:, 0:4], in0=gsb[:, 0:4],
                                        scalar1=-1.0, scalar2=None, op0=OP.mult)
                nc.vector.tensor_tensor(out=sc8[:, 0:4], in0=gsb[:, 0:4],
                                        in1=gsb[:, 0:4], op=OP.mult)
                nc.vector.tensor_tensor(out=sc8[:, 4:8], in0=gsb[:, 4:8],
                                        in1=sc8[:, 0:4], op=OP.subtract)
                nc.scalar.activation(out=sc8[:, 0:4], in_=sc8[:, 4:8], func=AF.Ln,
                                     bias=eps_t[0:8])
                nc.scalar.activation(out=nmr[:, 4:8], in_=sc8[:, 0:4], func=AF.Exp,
                                     scale=-0.5)
                xn_tiles, xhat_tiles = [], []
                sums = ptile([1, nfree], "op")
                sumsq = ptile([1, nfree], "op")
                oc = ones_col if xn_dtype == f32 else ones_colb
                for c in range(CH):
                    mexp = ptile([128, 2], "sp")
                    nc.tensor.matmul(mexp[:], e8_t[:], nmr[:, c:c + 5:4],
                                     start=True, stop=True)
                    stl = stile([128, 2], f32, "ss")
                    nc.vector.tensor_tensor(out=stl[:, 1:2], in0=mexp[:, 1:2],
                                            in1=gw[gidx][c][:], op=OP.mult)
                    nc.vector.scalar_tensor_tensor(out=stl[:, 0:1], in0=mexp[:, 0:1],
                                                   scalar=stl[:, 1:2],
                                                   in1=gb[gidx][c][:],
                                                   op0=OP.mult, op1=OP.add)
                    if inplace:
                        xn_c = src[c]
                    else:
                        xn_c = stile([128, nfree], xn_dtype, xn_tag)
                    nc.vector.tensor_scalar(out=xn_c[:], in0=src[c][:],
                                            scalar1=stl[:, 1:2], scalar2=stl[:, 0:1],
                                            op0=OP.mult, op1=OP.add)
                    xn_tiles.append(xn_c)
                    sq2 = stile([128, nfree], f32, "sq")
                    nc.scalar.activation(out=sq2[:], in_=xn_c[:], func=AF.Square)
                    for off, w_ in halves(nfree):
                        nc.tensor.matmul(sums[0:1, off:off + w_], oc[:],
                                         xn_c[:, off:off + w_],
                                         start=(c == 0), stop=(c == CH - 1))
                        nc.tensor.matmul(sumsq[0:1, off:off + w_], ones_col[:],
                                         sq2[:, off:off + w_],
                                         start=(c == 0), stop=(c == CH - 1))
                r_nm = stile([1, nfree], f32, "row1")
                nc.vector.tensor_scalar(out=r_nm[:], in0=sums[:], scalar1=-1.0 / C,
                                        scalar2=None, op0=OP.mult)
                r_m2 = stile([1, nfree], f32, "row1")
                nc.vector.tensor_tensor(out=r_m2[:], in0=r_nm[:], in1=r_nm[:],
                                        op=OP.mult)
                r_va = stile([1, nfree], f32, "row1")
                nc.vector.scalar_tensor_tensor(out=r_va[:], in0=sumsq[:],
                                               scalar=1.0 / C, in1=r_m2[:],
                                               op0=OP.mult, op1=OP.subtract)
                r_ln = stile([1, nfree], f32, "row1")
                nc.scalar.activation(out=r_ln[:], in_=r_va[:], func=AF.Ln,
                                     bias=eps_t[0:1])
                r_A = stile([1, nfree], f32, "row1")
                nc.scalar.activation(out=r_A[:], in_=r_ln[:], func=AF.Exp,
                                     scale=-0.5)
                r_B = stile([1, nfree], f32, "row1")
                nc.vector.tensor_tensor(out=r_B[:], in0=r_nm[:], in1=r_A[:],
                                        op=OP.mult)
                a_b = ptile([128, nfree], "op")
                b_b = ptile([128, nfree], "op")
                for off, w_ in halves(nfree):
                    nc.tensor.matmul(a_b[:, off:off + w_], ones_r1[:],
                                     r_A[0:1, off:off + w_], start=True, stop=True)
                    nc.tensor.matmul(b_b[:, off:off + w_], ones_r1[:],
                                     r_B[0:1, off:off + w_], start=True, stop=True)
                for c in range(CH):
                    tmp = stile([128, nfree], f32, "sq")
                    nc.vector.tensor_tensor(out=tmp[:], in0=xn_tiles[c][:], in1=a_b[:],
                                            op=OP.mult)
                    xh_c = stile([128, nfree], bf, xhat_tag)
                    nc.vector.tensor_tensor(out=xh_c[:], in0=tmp[:], in1=b_b[:],
                                            op=OP.add)
                    xhat_tiles.append(xh_c)
                return xn_tiles, xhat_tiles

            # ---------------- projections ----------------
            def proj_fm(xh, wname, brow, nfree, out_tag):
                outs = []
                for mc in range(CH):
                    P = ptile([128, nfree], "op")
                    for off, w_ in halves(nfree):
                        nc.tensor.matmul(P[:, off:off + w_],
                                         bias_tiles[brow][0:1, mc * 128:(mc + 1) * 128],
                                         ones_bf[0:1, 0:w_], start=True, stop=False)
                        for kc in range(CH):
                            nc.tensor.matmul(P[:, off:off + w_],
                                             W[wname][kc][:, mc * 128:(mc + 1) * 128],
                                             xh[kc][:, off:off + w_],
                                             start=False, stop=(kc == CH - 1))
                    o = stile([128, nfree], bf, out_tag)
                    nc.vector.tensor_copy(o[:], P[:])
                    outs.append(o)
                return outs

            def proj_v(xh, wname, brow, ntok, tag):
                vts = []
                for tcn in range((ntok + 127) // 128):
                    rows = min(128, ntok - tcn * 128)
                    P = ptile([128, 512], "op")
                    nc.tensor.matmul(P[0:rows, :], ones_bf[0:1, 0:rows],
                                     bias_tiles[brow][0:1, 0:512], start=True,
                                     stop=False)
                    for kc in range(CH):
                        nc.tensor.matmul(P[0:rows, :],
                                         xh[kc][:, tcn * 128:tcn * 128 + rows],
                                         W[wname][kc][:, 0:512],
                                         start=False, stop=(kc == CH - 1))
                    vt = stile([128, NH * (HD + 1)], bf, tag)
                    v3 = vt.rearrange("p (h x) -> p h x", x=HD + 1)
                    nc.vector.memset(v3[0:rows, :, HD:HD + 1], 1.0)
                    nc.vector.tensor_copy(v3[0:rows, :, 0:HD],
                                          P[0:rows, :].rearrange("p (h x) -> p h x",
                                                                 x=HD))
                    vts.append(vt)
                return vts

            # ---------------- attention ----------------
            def attention(qt, kmap, vmap, nkeys):
                onorms = []
                nkc = len(kmap)
                if ATTP < 1:
                    for hp in range(CH):
                        op_z = stile([128, HW], bf, "onorm")
                        nc.vector.memset(op_z[:], 0.0)
                        onorms.append(op_z)
                    return onorms
                for hp in range(CH):
                    Oa = ptile([128, HW], "op")
                    Ob = ptile([128, HW], "op")
                    ha, hb = 2 * hp, 2 * hp + 1
                    for g0 in range(0, nkc, KCG):
                        grp = range(g0, min(g0 + KCG, nkc))
                        etiles = {}
                        for kc in grp:
                            ktiles, koff = kmap[kc]
                            rows = nkeys[kc]
                            S = ptile([128, 2 * SOFF], "sp")
                            for off, w_ in halves(HW):
                                nc.tensor.matmul(
                                    S[0:rows, off:off + w_],
                                    ktiles[hp][0:64, koff:koff + rows],
                                    qt[hp][0:64, off:off + w_],
                                    start=True, stop=True, tile_position=(0, 0))
                                nc.tensor.matmul(
                                    S[0:rows, SOFF + off:SOFF + off + w_],
                                    ktiles[hp][64:128, koff:koff + rows],
                                    qt[hp][64:128, off:off + w_],
                                    start=True, stop=True, tile_position=(64, 0))
                            e = stile([128, 2 * HW], bf, "expS")
                            if SOFF == HW:
                                nc.scalar.activation(out=e[0:rows, :],
                                                     in_=S[0:rows, :], func=AF.Exp)
                            else:
                                nc.scalar.activation(out=e[0:rows, 0:HW],
                                                     in_=S[0:rows, 0:HW], func=AF.Exp)
                                nc.scalar.activation(out=e[0:rows, HW:2 * HW],
                                                     in_=S[0:rows, SOFF:SOFF + HW],
                                                     func=AF.Exp)
                            etiles[kc] = e
                        for kc in (grp if ATTP >= 2 else []):
                            vt = vmap[kc]
                            rows = nkeys[kc]
                            e = etiles[kc]
                            st_, sp_ = (kc == 0), (kc == nkc - 1)
                            for off, w_ in halves(HW):
                                nc.tensor.matmul(
                                    Oa[0:65, off:off + w_],
                                    vt[0:rows, (HD + 1) * ha:(HD + 1) * (ha + 1)],
                                    e[0:rows, off:off + w_],
                                    start=st_, stop=sp_)
                                nc.tensor.matmul(
                                    Ob[0:65, off:off + w_],
                                    vt[0:rows, (HD + 1) * hb:(HD + 1) * (hb + 1)],
                                    e[0:rows, HW + off:HW + off + w_],
                                    start=st_, stop=sp_)
                    if ATTP < 2:
                        op_z = stile([128, HW], bf, "onorm")
                        nc.vector.memset(op_z[:], 0.0)
                        onorms.append(op_z)
                        continue
                    if ATTP < 3:
                        op_z = stile([128, HW], bf, "onorm")
                        nc.vector.tensor_copy(op_z[0:64, :], Oa[0:64, :])
                        nc.vector.tensor_copy(op_z[64:128, :], Ob[0:64, :])
                        onorms.append(op_z)
                        continue
                    den_a = stile([1, HW], f32, "row1")
                    den_b = stile([1, HW], f32, "row1")
                    nc.vector.tensor_copy(den_a[:], Oa[64:65, :])
                    nc.vector.tensor_copy(den_b[:], Ob[64:65, :])
                    rec_a = stile([1, HW], f32, "row1")
                    rec_b = stile([1, HW], f32, "row1")
                    scr_a = stile([1, HW], f32, "row1")
                    scr_b = stile([1, HW], f32, "row1")
                    nc.vector.reciprocal_approx_accurate(rec_a[:], den_a[:], scr_a[:])
                    nc.vector.reciprocal_approx_accurate(rec_b[:], den_b[:], scr_b[:])
                    rb = ptile([128, HW], "sp")
                    for off, w_ in halves(HW):
                        nc.tensor.matmul(rb[0:64, off:off + w_], ones_r1[0:1, 0:64],
                                         rec_a[0:1, off:off + w_],
                                         start=True, stop=True, tile_position=(0, 0))
                        nc.tensor.matmul(rb[64:128, off:off + w_], ones_r1[0:1, 0:64],
                                         rec_b[0:1, off:off + w_],
                                         start=True, stop=True, tile_position=(0, 64))
                    rbs = stile([128, HW], f32, "rbs")
                    nc.vector.tensor_copy(rbs[:], rb[:])
                    o_p = stile([128, HW], bf, "onorm")
                    nc.vector.tensor_tensor(out=o_p[0:64, :], in0=Oa[0:64, :],
                                            in1=rbs[0:64, :], op=OP.mult)
                    nc.vector.tensor_tensor(out=o_p[64:128, :], in0=Ob[0:64, :],
                                            in1=rbs[64:128, :], op=OP.mult)
                    onorms.append(o_p)
                return onorms

            def out_proj(onorms, wname, brow, dname, xh, sink):
                for mc in range(CH):
                    P1 = ptile([128, HW], "op")
                    P2 = ptile([128, HW], "op")
                    for off, w_ in halves(HW):
                        nc.tensor.matmul(P1[:, off:off + w_],
                                         bias_tiles[brow][0:1, mc * 128:(mc + 1) * 128],
                                         ones_bf[0:1, 0:w_], start=True, stop=False)
                        for h8 in range(0, NH, 2):
                            lw = W[wname][h8 // 2][0:64, mc * 128:(mc + 1) * 128]
                            nc.tensor.matmul(P1[:, off:off + w_], lw,
                                             onorms[h8 // 2][0:64, off:off + w_],
                                             start=False, stop=False)
                        nc.tensor.matmul(P1[:, off:off + w_], W[dname][mc][:],
                                         xh[mc][:, off:off + w_],
                                         start=False, stop=True)
                        for i, h8 in enumerate(range(1, NH, 2)):
                            lw = W[wname][h8 // 2][64:128, mc * 128:(mc + 1) * 128]
                            nc.tensor.matmul(P2[:, off:off + w_], lw,
                                             onorms[h8 // 2][64:128, off:off + w_],
                                             start=(i == 0), stop=(i == 3))
                    sink(mc, P1, P2)

            # ---------------- ctx prep ----------------
            ctx_k, ctx_v = [], []
            for r in range(2):
                csrc = []
                for c in range(CH):
                    t = stile([128, NT], f32, "ctxin")
                    nc.sync.dma_start(out=t[:], in_=ctxin[r, c])
                    csrc.append(t)
                sums = ptile([1, NT], "op")
                sumsq = ptile([1, NT], "op")
                for c in range(CH):
                    sq2 = stile([128, NT], f32, "ctxin")
                    nc.scalar.activation(out=sq2[:], in_=csrc[c][:], func=AF.Square)
                    nc.tensor.matmul(sums[0:1, :], ones_col[:], csrc[c][:],
                                     start=(c == 0), stop=(c == CH - 1))
                    nc.tensor.matmul(sumsq[0:1, :], ones_col[:], sq2[:],
                                     start=(c == 0), stop=(c == CH - 1))
                r_nm = stile([1, NT], f32, "ctxr")
                nc.vector.tensor_scalar(out=r_nm[:], in0=sums[:], scalar1=-1.0 / C,
                                        scalar2=None, op0=OP.mult)
                r_m2 = stile([1, NT], f32, "ctxr")
                nc.vector.tensor_tensor(out=r_m2[:], in0=r_nm[:], in1=r_nm[:],
                                        op=OP.mult)
                r_va = stile([1, NT], f32, "ctxr")
                nc.vector.scalar_tensor_tensor(out=r_va[:], in0=sumsq[:],
                                               scalar=1.0 / C, in1=r_m2[:],
                                               op0=OP.mult, op1=OP.subtract)
                r_ln = stile([1, NT], f32, "ctxr")
                nc.scalar.activation(out=r_ln[:], in_=r_va[:], func=AF.Ln,
                                     bias=eps_t[0:1])
                r_A = stile([1, NT], f32, "ctxr")
                nc.scalar.activation(out=r_A[:], in_=r_ln[:], func=AF.Exp,
                                     scale=-0.5)
                r_B = stile([1, NT], f32, "ctxr")
                nc.vector.tensor_tensor(out=r_B[:], in0=r_nm[:], in1=r_A[:],
                                        op=OP.mult)
                a_b = ptile([128, NT], "op")
                b_b = ptile([128, NT], "op")
                nc.tensor.matmul(a_b[:], ones_r1[:], r_A[0:1, :], start=True, stop=True)
                nc.tensor.matmul(b_b[:], ones_r1[:], r_B[0:1, :], start=True, stop=True)
                ch_tiles = []
                for c in range(CH):
                    tmp = stile([128, NT], f32, "ctxin")
                    nc.vector.tensor_tensor(out=tmp[:], in0=csrc[c][:], in1=a_b[:],
                                            op=OP.mult)
                    xh_c = stile([128, NT], bf, "ctxh")
                    nc.vector.tensor_tensor(out=xh_c[:], in0=tmp[:], in1=b_b[:],
                                            op=OP.add)
                    ch_tiles.append(xh_c)
                ctx_k.append(proj_fm(ch_tiles, "cawk", 5, NT, "kctx"))
                ctx_v.append(proj_v(ch_tiles, "cawv", 6, NT, "vctx"))

            # ---------------- per-frame flow ----------------
            frames = {}

            def prep(fi, need_q):
                src = []
                for c in range(CH):
                    t = stile([128, HW], f32, "xin")
                    nc.sync.dma_start(out=t[:], in_=xin[fi, c])
                    src.append(t)
                xn, xh = norm_block(src, 0, "xnb", bf, HW, "xhat")
                d = {"xn": xn, "xh": xh}
                d["k"] = proj_fm(xh, "wk", 1, HW, "kT")
                d["v"] = proj_v(xh, "wv", 2, HW, "vtok")
                if need_q:
                    d["q"] = proj_fm(xh, "wq", 0, HW, "q")
                frames[fi] = d

            def self_block(fi):
                fr = frames[fi]
                pv = frames[fi - 1]
                kmap, vmap, nkeys = [], [], []
                for kc in range(NKC):
                    fsel = pv if kc < NTC else fr
                    kmap.append((fsel["k"], (kc % NTC) * 128))
                    vmap.append(fsel["v"][kc % NTC])
                    nkeys.append(128)
                onorms = attention(fr["q"], kmap, vmap, nkeys)
                xs2 = []

                def sink(mc, P1, P2):
                    t_c = stile([128, HW], f32, "sq")
                    nc.vector.tensor_tensor(out=t_c[:], in0=fr["xn"][mc][:],
                                            in1=P1[:], op=OP.add)
                    xs2_c = stile([128, HW], f32, "fp")
                    nc.vector.tensor_tensor(out=xs2_c[:], in0=t_c[:],
                                            in1=P2[:], op=OP.add)
                    xs2.append(xs2_c)

                out_proj(onorms, "wo", 3, "diag", fr["xh"], sink)
                return xs2

            def cross_block(fi, xs2):
                r = (fi - 1) % 2
                v2, xh2 = norm_block(xs2, 1, "fp", f32, HW, "xh2", inplace=True)
                q2 = proj_fm(xh2, "cawq", 4, HW, "q")
                onorms = attention(q2, [(ctx_k[r], 0)], [ctx_v[r][0]], [NT])

                def sink(mc, P1, P2):
                    t_c = stile([128, HW], f32, "sq")
                    nc.vector.tensor_copy(t_c[:], P1[:])
                    fin = stile([128, HW], f32, "sq")
                    nc.vector.tensor_tensor(out=fin[:], in0=t_c[:], in1=P2[:],
                                            op=OP.add)
                    nc.sync.dma_start(out=outD[fi - 1, mc], in_=fin[:])

                out_proj(onorms, "cawo", 7, "cadiag", xh2, sink)

            if PHASES < 99:
                z = stile([128, HW], f32, "sq")
                nc.vector.memset(z[:], 0.0)
                for fi in range(2):
                    for mc in range(CH):
                        nc.sync.dma_start(out=outD[fi, mc], in_=z[:])
            if PHASES >= 2:
                prep(0, need_q=False)
                prep(1, need_q=True)
            if PHASES >= 3:
                xs2_1 = self_block(1)
            if PHASES >= 4:
                cross_block(1, xs2_1)
            if PHASES >= 5:
                prep(2, need_q=True)
                cross_block(2, self_block(2))

    nc.compile()
    return nc


# ---------------------------------------------------------------------------
# host side: weight folding, sharding, assembly
# ---------------------------------------------------------------------------

def fold_weights(inp):
    hd_s = HD ** -0.5
    w = {}
    wv_, bv_ = inp['sa_lnv_w'], inp['sa_lnv_b']
    wl_, bl_ = inp['sa_lnl_w'], inp['sa_lnl_b']
    w['wq'] = (inp['sa_qw'] * wv_[None, :]).T * hd_s
    bq = (inp['sa_qw'] @ bv_ + inp['sa_qb']) * hd_s
    w['wk'] = (inp['sa_kw'] * wl_[None, :]).T
    bk = inp['sa_kw'] @ bl_ + inp['sa_kb']
    w['wv'] = (inp['sa_vw'] * wl_[None, :]).T
    bv2 = inp['sa_vw'] @ bl_ + inp['sa_vb']
    g = inp['sa_gamma']
    w['wo'] = (inp['sa_ow'] * g[:, None]).T
    bo = g * inp['sa_ob'] + bv_
    w['diag'] = wv_
    wv2_, bvv_ = inp['ca_lnv_w'], inp['ca_lnv_b']
    wl2_, bl2_ = inp['ca_lnl_w'], inp['ca_lnl_b']
    w['cawq'] = (inp['ca_qw'] * wv2_[None, :]).T * hd_s
    cbq = (inp['ca_qw'] @ bvv_ + inp['ca_qb']) * hd_s
    w['cawk'] = (inp['ca_kw'] * wl2_[None, :]).T
    cbk = inp['ca_kw'] @ bl2_ + inp['ca_kb']
    w['cawv'] = (inp['ca_vw'] * wl2_[None, :]).T
    cbv = inp['ca_vw'] @ bl2_ + inp['ca_vb']
    g2 = inp['ca_gamma']
    w['cawo'] = (inp['ca_ow'] * g2[:, None]).T
    cbo = g2 * inp['ca_ob'] + bvv_
    w['cadiag'] = wv2_
    bias = np.stack([bq, bk, bv2, bo, cbq, cbk, cbv, cbo]).astype(F32)
    return w, bias


def make_in_maps(inp, HW):
    x = inp['x'].reshape(B * T, C, HW)
    ctx_fm = np.ascontiguousarray(inp['context'].transpose(0, 2, 1))
    w, bias = fold_weights(inp)

    gnw = np.stack([inp['gn1_w'], inp['gn2_w']]).reshape(2, CH, 128, 1).astype(F32)
    gnb = np.stack([inp['gn1_b'], inp['gn2_b']]).reshape(2, CH, 128, 1).astype(F32)
    gsum = np.zeros((128, 8), F32)
    for p in range(128):
        gsum[p, p // 16] = 1.0 / (16 * HW)
    e8 = np.zeros((8, 128), F32)
    for p in range(128):
        e8[p // 16, p] = 1.0
    sel = np.zeros((2, 128), F32)
    sel[0, 0:64] = 1.0
    sel[1, 64:128] = 1.0

    common = {
        "ctxin": np.ascontiguousarray(ctx_fm.reshape(2, CH, 128, NT)),
        "gnw": gnw, "gnb": gnb, "gsum": gsum, "e8": e8, "sel": sel,
        "bias": bias.astype(BF16),
    }
    for name in ("wq", "wk", "wv", "wo", "cawq", "cawk", "cawv", "cawo"):
        common[name] = np.ascontiguousarray(
            w[name].astype(BF16).reshape(CH, 128, 512))
    for name, src in (("diag", "diag"), ("cadiag", "cadiag")):
        d4 = np.zeros((CH, 128, 128), F32)
        for c in range(CH):
            np.fill_diagonal(d4[c], w[src][c * 128:(c + 1) * 128])
        common[name] = d4.astype(BF16)

    in_maps = []
    for cid in range(N_CORES):
        b, j = cid // 4, cid % 4
        fA = 2 * j
        prev = max(fA - 1, 0)
        xloc = np.stack([x[b * T + prev], x[b * T + fA], x[b * T + fA + 1]])
        m = dict(common)
        m["xin"] = np.ascontiguousarray(xloc.reshape(3, CH, 128, HW))
        in_maps.append(m)
    return in_maps


def assemble(results, HW):
    out = np.empty((B * T, C, HW), F32)
    for cid in range(N_CORES):
        b, j = cid // 4, cid % 4
        o = results[cid]["out"]
        out[b * T + 2 * j] = o[0].reshape(C, HW)
        out[b * T + 2 * j + 1] = o[1].reshape(C, HW)
    H = int(round(np.sqrt(HW)))
    return out.reshape(B * T, C, H, H)


_CACHE = {}


def _get_module(HW=1024):
    if HW not in _CACHE:
        _CACHE[HW] = build_module(HW=HW)
    return _CACHE[HW]


def kernel(**inputs):
    from concourse.bass_utils import run_bass_kernel_spmd

    inp = {k: np.asarray(v, F32) for k, v in inputs.items()}
    HW = inp['x'].shape[2] * inp['x'].shape[3]
    nc = _get_module(HW)
    in_maps = make_in_maps(inp, HW)
    res = run_bass_kernel_spmd(nc, in_maps, core_ids=list(range(N_CORES)))
    return assemble(res.results, HW)


# revision 24
# speedup vs baseline: 3305.3570x; 3305.3570x over previous
# Trainium2 Bass kernel for nn_ExtendedSpatialAttention.
#
# Sharding: 16 (clip, frame) rows across 8 cores -> 2 frames per core
# (core c: clip b=c//4, frames 2j, 2j+1, j=c%4). Each core receives its two
# frames plus the 1-frame halo (frame 2j-1; frame 0 duplicated for j=0 --
# attention over a duplicated key set equals the single-frame window). No
# inter-core communication is needed.
#
# Device dataflow is feature-major ([C, tokens]); attention uses the
# "S-transposed" layout (keys on partitions): softmax denominators come from
# a ones-augmented V column in the PV matmul, so no transposes and no
# cross-partition reductions appear anywhere. LayerNorm affines are folded
# into projection weights on the host; softmax skips max-subtraction (scores
# are O(1), and the reference's global-max shift cancels mathematically).
import sys
import numpy as np

sys.path.insert(0, "/opt/trn_rl_repo")

import ml_dtypes

BF16 = ml_dtypes.bfloat16
F32 = np.float32
EPS = 1e-5
N_CORES = 8
C = 512
CH = 4            # channel chunks of 128
NH = 8            # heads
HD = 64           # head dim
T = 8             # frames per clip
B = 2             # clips
NT = 77           # text tokens


def build_module(HW=1024, KCG=2, PHASES=99, ATTP=99):
    import contextlib
    import concourse.bacc as bacc
    import concourse.mybir as mybir
    import concourse.tile as tile

    f32, bf = mybir.dt.float32, mybir.dt.bfloat16
    OP = mybir.AluOpType
    AF = mybir.ActivationFunctionType
    AX = mybir.AxisListType

    NTC = HW // 128
    NKC = 2 * NTC
    SOFF = max(HW, 512)

    # Route Exp/Ln/Square to the one ACT table set that contains all three
    # (natural_log_exp_and_others) so the kernel needs a single table load
    # instead of ping-ponging between the exp and ln sets (~2.7us per load).
    import concourse.hw_specs as hw_specs
    _special = {AF.Exp, AF.Ln, AF.Square}
    _tabs = hw_specs.get_activation_tables("gen3")
    for _name, _funcs in _tabs.items():
        if _name != "natural_log_exp_and_others" and "small" not in _name:
            _funcs -= _special

    nc = bacc.Bacc("TRN2", target_bir_lowering=False, debug=False,
                   enable_asserts=False, num_devices=N_CORES)

    xin = nc.dram_tensor("xin", [3, CH, 128, HW], f32, kind="ExternalInput").ap()
    ctxin = nc.dram_tensor("ctxin", [2, CH, 128, NT], f32, kind="ExternalInput").ap()
    outD = nc.dram_tensor("out", [2, CH, 128, HW], f32, kind="ExternalOutput").ap()
    gnwD = nc.dram_tensor("gnw", [2, CH, 128, 1], f32, kind="ExternalInput").ap()
    gnbD = nc.dram_tensor("gnb", [2, CH, 128, 1], f32, kind="ExternalInput").ap()
    gsumD = nc.dram_tensor("gsum", [128, 8], f32, kind="ExternalInput").ap()
    e8D = nc.dram_tensor("e8", [8, 128], f32, kind="ExternalInput").ap()
    selD = nc.dram_tensor("sel", [2, 128], f32, kind="ExternalInput").ap()
    biasD = nc.dram_tensor("bias", [8, 512], bf, kind="ExternalInput").ap()
    wD = {}
    for name in ("wq", "wk", "wv", "wo", "cawq", "cawk", "cawv", "cawo"):
        wD[name] = nc.dram_tensor(name, [CH, 128, 512], bf, kind="ExternalInput").ap()
    for name in ("diag", "cadiag"):
        wD[name] = nc.dram_tensor(name, [CH, 128, 128], bf, kind="ExternalInput").ap()

    with tile.TileContext(nc) as tc:
        with contextlib.ExitStack() as st:
            wp = st.enter_context(tc.tile_pool(name="wp", bufs=1))
            sp = st.enter_context(tc.tile_pool(name="spool", bufs=1))
            pp = st.enter_context(tc.tile_pool(name="ppool", bufs=1, space="PSUM"))

            BUFS = {
                "xin": 4, "sq": 2, "ss": 6, "nmr": 4, "xnb": 8, "fp": 4,
                "xhat": 7, "xh2": 4, "kT": 8, "vtok": 16, "vctx": 2, "q": 4,
                "expS": 2, "onorm": 4, "rbs": 1, "row1": 4, "ctxr": 6, "ctxin": 5,
                "ctxh": 8, "kctx": 8,
            }
            PBUFS = {"op": 2, "sp": 1}

            uid = [0]

            def nm(p):
                uid[0] += 1
                return f"{p}_{uid[0]}"

            def stile(shape, dtype, tag):
                return sp.tile(shape, dtype, name=nm(tag), tag=tag, bufs=BUFS[tag])

            def ptile(shape, tag):
                return pp.tile(shape, f32, name=nm(tag), tag=tag, bufs=PBUFS[tag])

            # ---------------- constants & weights ----------------
            W = {}
            for name in ("wq", "wk", "wv", "wo", "cawq", "cawk", "cawv", "cawo"):
                W[name] = []
                for c in range(CH):
                    t = wp.tile([128, 512], bf, name=f"{name}{c}")
                    nc.sync.dma_start(out=t[:], in_=wD[name][c])
                    W[name].append(t)
            for name in ("diag", "cadiag"):
                W[name] = []
                for c in range(CH):
                    t = wp.tile([128, 128], bf, name=f"{name}{c}")
                    nc.sync.dma_start(out=t[:], in_=wD[name][c])
                    W[name].append(t)
            gw, gb = [], []
            for g in range(2):
                gw.append([])
                gb.append([])
                for c in range(CH):
                    t = wp.tile([128, 1], f32, name=f"gw{g}{c}")
                    nc.sync.dma_start(out=t[:], in_=gnwD[g, c])
                    gw[g].append(t)
                    t2 = wp.tile([128, 1], f32, name=f"gb{g}{c}")
                    nc.sync.dma_start(out=t2[:], in_=gnbD[g, c])
                    gb[g].append(t2)
            gsum_t = wp.tile([128, 8], f32, name="gsum_t")
            nc.sync.dma_start(out=gsum_t[:], in_=gsumD[:])
            e8_t = wp.tile([8, 128], f32, name="e8_t")
            nc.sync.dma_start(out=e8_t[:], in_=e8D[:])
            bias_tiles = []
            for r in range(8):
                bt = wp.tile([1, 512], bf, name=f"bias{r}")
                nc.sync.dma_start(out=bt[:], in_=biasD[r:r + 1, :])
                bias_tiles.append(bt)
            ones_col = wp.tile([128, 1], f32, name="ones_col")
            nc.vector.memset(ones_col[:], 1.0)
            ones_colb = wp.tile([128, 1], bf, name="ones_colb")
            nc.vector.memset(ones_colb[:], 1.0)
            ones_r1 = wp.tile([1, 128], f32, name="ones_r1")
            nc.vector.memset(ones_r1[:], 1.0)
            ones_bf = wp.tile([1, 512], bf, name="ones_bf")
            nc.vector.memset(ones_bf[:], 1.0)
            eps_t = wp.tile([128, 1], f32, name="eps_t")
            nc.vector.memset(eps_t[:], EPS)

            def halves(nfree):
                return [(h * 512, 512) for h in range(nfree // 512)] or [(0, nfree)]

            # ---------------- GroupNorm + standardize-over-C ----------------
            def norm_block(src, gidx, xn_tag, xn_dtype, nfree, xhat_tag,
                           inplace=False):
                gstats = ptile([8, 8], "op")
                for c in range(CH):
                    sq = stile([128, nfree], f32, "sq")
                    ssum = stile([128, 2], f32, "ss")
                    nc.scalar.activation(out=sq[:], in_=src[c][:], func=AF.Square,
                                         accum_out=ssum[:, 1:2])
                    nc.vector.tensor_reduce(out=ssum[:, 0:1], in_=src[c][:],
                                            axis=AX.X, op=OP.add)
                    nc.tensor.matmul(gstats[0:8, c:c + 1], gsum_t[:, 0:8],
                                     ssum[:, 0:1], start=True, stop=True)
                    nc.tensor.matmul(gstats[0:8, 4 + c:5 + c], gsum_t[:, 0:8],
                                     ssum[:, 1:2], start=True, stop=True)
                gsb = stile([8, 8], f32, "nmr")
                nc.vector.tensor_copy(gsb[:], gstats[:])
                nmr = stile([8, 8], f32, "nmr")
                sc8 = stile([8, 8], f32, "nmr")
                nc.vector.tensor_scalar(out=nmr[:, 0:4], in0=gsb[:, 0:4],
                                        scalar1=-1.0, scalar2=None, op0=OP.mult)
                nc.vector.tensor_tensor(out=sc8[:, 0:4], in0=gsb[:, 0:4],
                                        in1=gsb[:, 0:4], op=OP.mult)
                nc.vector.tensor_tensor(out=sc8[:, 4:8], in0=gsb[:, 4:8],
                                        in1=sc8[:, 0:4], op=OP.subtract)
                nc.scalar.activation(out=sc8[:, 0:4], in_=sc8[:, 4:8], func=AF.Ln,
                                     bias=eps_t[0:8])
                nc.scalar.activation(out=nmr[:, 4:8], in_=sc8[:, 0:4], func=AF.Exp,
                                     scale=-0.5)
                xn_tiles, xhat_tiles = [], []
                sums = ptile([1, nfree], "op")
                sumsq = ptile([1, nfree], "op")
                oc = ones_col if xn_dtype == f32 else ones_colb
                for c in range(CH):
                    mexp = ptile([128, 2], "sp")
                    nc.tensor.matmul(mexp[:], e8_t[:], nmr[:, c:c + 5:4],
                                     start=True, stop=True)
                    stl = stile([128, 2], f32, "ss")
                    nc.vector.tensor_tensor(out=stl[:, 1:2], in0=mexp[:, 1:2],
                                            in1=gw[gidx][c][:], op=OP.mult)
                    nc.vector.scalar_tensor_tensor(out=stl[:, 0:1], in0=mexp[:, 0:1],
                                                   scalar=stl[:, 1:2],
                                                   in1=gb[gidx][c][:],
                                                   op0=OP.mult, op1=OP.add)
                    if inplace:
                        xn_c = src[c]
                    else:
                        xn_c = stile([128, nfree], xn_dtype, xn_tag)
                    nc.vector.tensor_scalar(out=xn_c[:], in0=src[c][:],
                                            scalar1=stl[:, 1:2], scalar2=stl[:, 0:1],
                                            op0=OP.mult, op1=OP.add)
                    xn_tiles.append(xn_c)
                    sq2 = stile([128, nfree], f32, "sq")
                    nc.scalar.activation(out=sq2[:], in_=xn_c[:], func=AF.Square)
                    for off, w_ in halves(nfree):
                        nc.tensor.matmul(sums[0:1, off:off + w_], oc[:],
                                         xn_c[:, off:off + w_],
                                         start=(c == 0), stop=(c == CH - 1))
                        nc.tensor.matmul(sumsq[0:1, off:off + w_], ones_col[:],
                                         sq2[:, off:off + w_],
                                         start=(c == 0), stop=(c == CH - 1))
                r_nm = stile([1, nfree], f32, "row1")
                nc.vector.tensor_scalar(out=r_nm[:], in0=sums[:], scalar1=-1.0 / C,
                                        scalar2=None, op0=OP.mult)
                r_m2 = stile([1, nfree], f32, "row1")
                nc.vector.tensor_tensor(out=r_m2[:], in0=r_nm[:], in1=r_nm[:],
                                        op=OP.mult)
                r_va = stile([1, nfree], f32, "row1")
                nc.vector.scalar_tensor_tensor(out=r_va[:], in0=sumsq[:],
                                               scalar=1.0 / C, in1=r_m2[:],
                                               op0=OP.mult, op1=OP.subtract)
                r_ln = stile([1, nfree], f32, "row1")
                nc.scalar.activation(out=r_ln[:], in_=r_va[:], func=AF.Ln,
                                     bias=eps_t[0:1])
                r_A = stile([1, nfree], f32, "row1")
                nc.scalar.activation(out=r_A[:], in_=r_ln[:], func=AF.Exp,
                                     scale=-0.5)
                r_B = stile([1, nfree], f32, "row1")
                nc.vector.tensor_tensor(out=r_B[:], in0=r_nm[:], in1=r_A[:],
                                        op=OP.mult)
                a_b = ptile([128, nfree], "op")
                b_b = ptile([128, nfree], "op")
                for off, w_ in halves(nfree):
                    nc.tensor.matmul(a_b[:, off:off + w_], ones_r1[:],
                                     r_A[0:1, off:off + w_], start=True, stop=True)
                    nc.tensor.matmul(b_b[:, off:off + w_], ones_r1[:],
                                     r_B[0:1, off:off + w_], start=True, stop=True)
                for c in range(CH):
                    tmp = stile([128, nfree], f32, "sq")
                    nc.vector.tensor_tensor(out=tmp[:], in0=xn_tiles[c][:], in1=a_b[:],
                                            op=OP.mult)
                    xh_c = stile([128, nfree], bf, xhat_tag)
                    nc.vector.tensor_tensor(out=xh_c[:], in0=tmp[:], in1=b_b[:],
                                            op=OP.add)
                    xhat_tiles.append(xh_c)
                return xn_tiles, xhat_tiles

            # ---------------- projections ----------------
            def proj_fm(xh, wname, brow, nfree, out_tag):
                outs = []
                for mc in range(CH):
                    P = ptile([128, nfree], "op")
                    for off, w_ in halves(nfree):
                        nc.tensor.matmul(P[:, off:off + w_],
                                         bias_tiles[brow][0:1, mc * 128:(mc + 1) * 128],
                                         ones_bf[0:1, 0:w_], start=True, stop=False)
                        for kc in range(CH):
                            nc.tensor.matmul(P[:, off:off + w_],
                                             W[wname][kc][:, mc * 128:(mc + 1) * 128],
                                             xh[kc][:, off:off + w_],
                                             start=False, stop=(kc == CH - 1))
                    o = stile([128, nfree], bf, out_tag)
                    nc.vector.tensor_copy(o[:], P[:])
                    outs.append(o)
                return outs

            def proj_v(xh, wname, brow, ntok, tag):
                vts = []
                for tcn in range((ntok + 127) // 128):
                    rows = min(128, ntok - tcn * 128)
                    P = ptile([128, 512], "op")
                    nc.tensor.matmul(P[0:rows, :], ones_bf[0:1, 0:rows],
                                     bias_tiles[brow][0:1, 0:512], start=True,
                                     stop=False)
                    for kc in range(CH):
                        nc.tensor.matmul(P[0:rows, :],
                                         xh[kc][:, tcn * 128:tcn * 128 + rows],
                                         W[wname][kc][:, 0:512],
                                         start=False, stop=(kc == CH - 1))
                    vt = stile([128, NH * (HD + 1)], bf, tag)
                    v3 = vt.rearrange("p (h x) -> p h x", x=HD + 1)
                    nc.vector.memset(v3[0:rows, :, HD:HD + 1], 1.0)
                    nc.vector.tensor_copy(v3[0:rows, :, 0:HD],
                                          P[0:rows, :].rearrange("p (h x) -> p h x",
                                                                 x=HD))
                    vts.append(vt)
                return vts

            # ---------------- attention ----------------
            def attention(qt, kmap, vmap, nkeys):
                onorms = []
                nkc = len(kmap)
                if ATTP < 1:
                    for hp in range(CH):
                        op_z = stile([128, HW], bf, "onorm")
                        nc.vector.memset(op_z[:], 0.0)
                        onorms.append(op_z)
                    return onorms
                for hp in range(CH):
                    Oa = ptile([128, HW], "op")
                    Ob = ptile([128, HW], "op")
                    ha, hb = 2 * hp, 2 * hp + 1
                    for g0 in range(0, nkc, KCG):
                        grp = range(g0, min(g0 + KCG, nkc))
                        etiles = {}
                        for kc in grp:
                            ktiles, koff = kmap[kc]
                            rows = nkeys[kc]
                            S = ptile([128, 2 * SOFF], "sp")
                            for off, w_ in halves(HW):
                                nc.tensor.matmul(
                                    S[0:rows, off:off + w_],
                                    ktiles[hp][0:64, koff:koff + rows],
                                    qt[hp][0:64, off:off + w_],
                                    start=True, stop=True, tile_position=(0, 0))
                                nc.tensor.matmul(
                                    S[0:rows, SOFF + off:SOFF + off + w_],
                                    ktiles[hp][64:128, koff:koff + rows],
                                    qt[hp][64:128, off:off + w_],
                                    start=True, stop=True, tile_position=(64, 0))
                            e = stile([128, 2 * HW], bf, "expS")
                            if SOFF == HW:
                                nc.scalar.activation(out=e[0:rows, :],
                                                     in_=S[0:rows, :], func=AF.Exp)
                            else:
                                nc.scalar.activation(out=e[0:rows, 0:HW],
                                                     in_=S[0:rows, 0:HW], func=AF.Exp)
                                nc.scalar.activation(out=e[0:rows, HW:2 * HW],
                                                     in_=S[0:rows, SOFF:SOFF + HW],
                                                     func=AF.Exp)
                            etiles[kc] = e
                        for kc in (grp if ATTP >= 2 else []):
                            vt = vmap[kc]
                            rows = nkeys[kc]
                            e = etiles[kc]
                            st_, sp_ = (kc == 0), (kc == nkc - 1)
                            for off, w_ in halves(HW):
                                nc.tensor.matmul(
                                    Oa[0:65, off:off + w_],
                                    vt[0:rows, (HD + 1) * ha:(HD + 1) * (ha + 1)],
                                    e[0:rows, off:off + w_],
                                    start=st_, stop=sp_)
                                nc.tensor.matmul(
                                    Ob[0:65, off:off + w_],
                                    vt[0:rows, (HD + 1) * hb:(HD + 1) * (hb + 1)],
                                    e[0:rows, HW + off:HW + off + w_],
                                    start=st_, stop=sp_)
                    if ATTP < 2:
                        op_z = stile([128, HW], bf, "onorm")
                        nc.vector.memset(op_z[:], 0.0)
                        onorms.append(op_z)
                        continue
                    if ATTP < 3:
                        op_z = stile([128, HW], bf, "onorm")
                        nc.vector.tensor_copy(op_z[0:64, :], Oa[0:64, :])
                        nc.vector.tensor_copy(op_z[64:128, :], Ob[0:64, :])
                        onorms.append(op_z)
                        continue
                    den_a = stile([1, HW], f32, "row1")
                    den_b = stile([1, HW], f32, "row1")
                    nc.vector.tensor_copy(den_a[:], Oa[64:65, :])
                    nc.vector.tensor_copy(den_b[:], Ob[64:65, :])
                    rec_a = stile([1, HW], f32, "row1")
                    rec_b = stile([1, HW], f32, "row1")
                    scr_a = stile([1, HW], f32, "row1")
                    scr_b = stile([1, HW], f32, "row1")
                    nc.vector.reciprocal_approx_accurate(rec_a[:], den_a[:], scr_a[:])
                    nc.vector.reciprocal_approx_accurate(rec_b[:], den_b[:], scr_b[:])
                    rb = ptile([128, HW], "sp")
                    for off, w_ in halves(HW):
                        nc.tensor.matmul(rb[0:64, off:off + w_], ones_r1[0:1, 0:64],
                                         rec_a[0:1, off:off + w_],
                                         start=True, stop=True, tile_position=(0, 0))
                        nc.tensor.matmul(rb[64:128, off:off + w_], ones_r1[0:1, 0:64],
                                         rec_b[0:1, off:off + w_],
                                         start=True, stop=True, tile_position=(0, 64))
                    rbs = stile([128, HW], f32, "rbs")
                    nc.vector.tensor_copy(rbs[:], rb[:])
                    o_p = stile([128, HW], bf, "onorm")
                    nc.vector.tensor_tensor(out=o_p[0:64, :], in0=Oa[0:64, :],
                                            in1=rbs[0:64, :], op=OP.mult)
                    nc.vector.tensor_tensor(out=o_p[64:128, :], in0=Ob[0:64, :],
                                            in1=rbs[64:128, :], op=OP.mult)
                    onorms.append(o_p)
                return onorms

            def out_proj(onorms, wname, brow, dname, xh, sink):
                for mc in range(CH):
                    P1 = ptile([128, HW], "op")
                    P2 = ptile([128, HW], "op")
                    for off, w_ in halves(HW):
                        nc.tensor.matmul(P1[:, off:off + w_],
                                         bias_tiles[brow][0:1, mc * 128:(mc + 1) * 128],
                                         ones_bf[0:1, 0:w_], start=True, stop=False)
                        for h8 in range(0, NH, 2):
                            lw = W[wname][h8 // 2][0:64, mc * 128:(mc + 1) * 128]
                            nc.tensor.matmul(P1[:, off:off + w_], lw,
                                             onorms[h8 // 2][0:64, off:off + w_],
                                             start=False, stop=False)
                        nc.tensor.matmul(P1[:, off:off + w_], W[dname][mc][:],
                                         xh[mc][:, off:off + w_],
                                         start=False, stop=True)
                        for i, h8 in enumerate(range(1, NH, 2)):
                            lw = W[wname][h8 // 2][64:128, mc * 128:(mc + 1) * 128]
                            nc.tensor.matmul(P2[:, off:off + w_], lw,
                                             onorms[h8 // 2][64:128, off:off + w_],
                                             start=(i == 0), stop=(i == 3))
                    sink(mc, P1, P2)

            # ---------------- ctx prep ----------------
            ctx_k, ctx_v = [], []
            for r in range(2):
                csrc = []
                for c in range(CH):
                    t = stile([128, NT], f32, "ctxin")
                    nc.sync.dma_start(out=t[:], in_=ctxin[r, c])
                    csrc.append(t)
                sums = ptile([1, NT], "op")
                sumsq = ptile([1, NT], "op")
                for c in range(CH):
                    sq2 = stile([128, NT], f32, "ctxin")
                    nc.scalar.activation(out=sq2[:], in_=csrc[c][:], func=AF.Square)
                    nc.tensor.matmul(sums[0:1, :], ones_col[:], csrc[c][:],
                                     start=(c == 0), stop=(c == CH - 1))
                    nc.tensor.matmul(sumsq[0:1, :], ones_col[:], sq2[:],
                                     start=(c == 0), stop=(c == CH - 1))
                r_nm = stile([1, NT], f32, "ctxr")
                nc.vector.tensor_scalar(out=r_nm[:], in0=sums[:], scalar1=-1.0 / C,
                                        scalar2=None, op0=OP.mult)
                r_m2 = stile([1, NT], f32, "ctxr")
                nc.vector.tensor_tensor(out=r_m2[:], in0=r_nm[:], in1=r_nm[:],
                                        op=OP.mult)
                r_va = stile([1, NT], f32, "ctxr")
                nc.vector.scalar_tensor_tensor(out=r_va[:], in0=sumsq[:],
                                               scalar=1.0 / C, in1=r_m2[:],
                                               op0=OP.mult, op1=OP.subtract)
                r_ln = stile([1, NT], f32, "ctxr")
                nc.scalar.activation(out=r_ln[:], in_=r_va[:], func=AF.Ln,
                                     bias=eps_t[0:1])
                r_A = stile([1, NT], f32, "ctxr")
                nc.scalar.activation(out=r_A[:], in_=r_ln[:], func=AF.Exp,
                                     scale=-0.5)
                r_B = stile([1, NT], f32, "ctxr")
                nc.vector.tensor_tensor(out=r_B[:], in0=r_nm[:], in1=r_A[:],
                                        op=OP.mult)
                a_b = ptile([128, NT], "op")
                b_b = ptile([128, NT], "op")
                nc.tensor.matmul(a_b[:], ones_r1[:], r_A[0:1, :], start=True, stop=True)
                nc.tensor.matmul(b_b[:], ones_r1[:], r_B[0:1, :], start=True, stop=True)
                ch_tiles = []
                for c in range(CH):
                    tmp = stile([128, NT], f32, "ctxin")
                    nc.vector.tensor_tensor(out=tmp[:], in0=csrc[c][:], in1=a_b[:],
                                            op=OP.mult)
                    xh_c = stile([128, NT], bf, "ctxh")
                    nc.vector.tensor_tensor(out=xh_c[:], in0=tmp[:], in1=b_b[:],
                                            op=OP.add)
                    ch_tiles.append(xh_c)
                ctx_k.append(proj_fm(ch_tiles, "cawk", 5, NT, "kctx"))
                ctx_v.append(proj_v(ch_tiles, "cawv", 6, NT, "vctx"))

            # ---------------- per-frame flow ----------------
            frames = {}

            def prep(fi, need_q):
                src = []
                for c in range(CH):
                    t = stile([128, HW], f32, "xin")
                    nc.sync.dma_start(out=t[:], in_=xin[fi, c])
                    src.append(t)
                xn, xh = norm_block(src, 0, "xnb", bf, HW, "xhat")
                d = {"xn": xn, "xh": xh}
                d["k"] = proj_fm(xh, "wk", 1, HW, "kT")
                d["v"] = proj_v(xh, "wv", 2, HW, "vtok")
                if need_q:
                    d["q"] = proj_fm(xh, "wq", 0, HW, "q")
                frames[fi] = d

            def self_block(fi):
                fr = frames[fi]
                pv = frames[fi - 1]
                kmap, vmap, nkeys = [], [], []
                for kc in range(NKC):
                    fsel = pv if kc < NTC else fr
                    kmap.append((fsel["k"], (kc % NTC) * 128))
                    vmap.append(fsel["v"][kc % NTC])
                    nkeys.append(128)
                onorms = attention(fr["q"], kmap, vmap, nkeys)
                xs2 = []

                def sink(mc, P1, P2):
                    t_c = stile([128, HW], f32, "sq")
                    nc.vector.tensor_tensor(out=t_c[:], in0=fr["xn"][mc][:],
                                            in1=P1[:], op=OP.add)
                    xs2_c = stile([128, HW], f32, "fp")
                    nc.vector.tensor_tensor(out=xs2_c[:], in0=t_c[:],
                                            in1=P2[:], op=OP.add)
                    xs2.append(xs2_c)

                out_proj(onorms, "wo", 3, "diag", fr["xh"], sink)
                return xs2

            def cross_block(fi, xs2):
                r = (fi - 1) % 2
                v2, xh2 = norm_block(xs2, 1, "fp", f32, HW, "xh2", inplace=True)
                q2 = proj_fm(xh2, "cawq", 4, HW, "q")
                onorms = attention(q2, [(ctx_k[r], 0)], [ctx_v[r][0]], [NT])

                def sink(mc, P1, P2):
                    t_c = stile([128, HW], f32, "sq")
                    nc.vector.tensor_copy(t_c[:], P1[:])
                    fin = stile([128, HW], f32, "sq")
                    nc.vector.tensor_tensor(out=fin[:], in0=t_c[:], in1=P2[:],
                                            op=OP.add)
                    nc.sync.dma_start(out=outD[fi - 1, mc], in_=fin[:])

                out_proj(onorms, "cawo", 7, "cadiag", xh2, sink)

            if PHASES < 99:
                z = stile([128, HW], f32, "sq")
                nc.vector.memset(z[:], 0.0)
                for fi in range(2):
                    for mc in range(CH):
                        nc.sync.dma_start(out=outD[fi, mc], in_=z[:])
            if PHASES >= 2:
                prep(0, need_q=False)
                prep(1, need_q=True)
            if PHASES >= 3:
                xs2_1 = self_block(1)
            if PHASES >= 4:
                cross_block(1, xs2_1)
            if PHASES >= 5:
                prep(2, need_q=True)
                cross_block(2, self_block(2))

    nc.compile()
    return nc


# ---------------------------------------------------------------------------
# host side: weight folding, sharding, assembly
# ---------------------------------------------------------------------------

def fold_weights(inp):
    hd_s = HD ** -0.5
    w = {}
    wv_, bv_ = inp['sa_lnv_w'], inp['sa_lnv_b']
    wl_, bl_ = inp['sa_lnl_w'], inp['sa_lnl_b']
    w['wq'] = (inp['sa_qw'] * wv_[None, :]).T * hd_s
    bq = (inp['sa_qw'] @ bv_ + inp['sa_qb']) * hd_s
    w['wk'] = (inp['sa_kw'] * wl_[None, :]).T
    bk = inp['sa_kw'] @ bl_ + inp['sa_kb']
    w['wv'] = (inp['sa_vw'] * wl_[None, :]).T
    bv2 = inp['sa_vw'] @ bl_ + inp['sa_vb']
    g = inp['sa_gamma']
    w['wo'] = (inp['sa_ow'] * g[:, None]).T
    bo = g * inp['sa_ob'] + bv_
    w['diag'] = wv_
    wv2_, bvv_ = inp['ca_lnv_w'], inp['ca_lnv_b']
    wl2_, bl2_ = inp['ca_lnl_w'], inp['ca_lnl_b']
    w['cawq'] = (inp['ca_qw'] * wv2_[None, :]).T * hd_s
    cbq = (inp['ca_qw'] @ bvv_ + inp['ca_qb']) * hd_s
    w['cawk'] = (inp['ca_kw'] * wl2_[None, :]).T
    cbk = inp['ca_kw'] @ bl2_ + inp['ca_kb']
    w['cawv'] = (inp['ca_vw'] * wl2_[None, :]).T
    cbv = inp['ca_vw'] @ bl2_ + inp['ca_vb']
    g2 = inp['ca_gamma']
    w['cawo'] = (inp['ca_ow'] * g2[:, None]).T
    cbo = g2 * inp['ca_ob'] + bvv_
    w['cadiag'] = wv2_
    bias = np.stack([bq, bk, bv2, bo, cbq, cbk, cbv, cbo]).astype(F32)
    return w, bias


def make_in_maps(inp, HW):
    x = inp['x'].reshape(B * T, C, HW)
    ctx_fm = np.ascontiguousarray(inp['context'].transpose(0, 2, 1))
    w, bias = fold_weights(inp)

    gnw = np.stack([inp['gn1_w'], inp['gn2_w']]).reshape(2, CH, 128, 1).astype(F32)
    gnb = np.stack([inp['gn1_b'], inp['gn2_b']]).reshape(2, CH, 128, 1).astype(F32)
    gsum = np.zeros((128, 8), F32)
    for p in range(128):
        gsum[p, p // 16] = 1.0 / (16 * HW)
    e8 = np.zeros((8, 128), F32)
    for p in range(128):
        e8[p // 16, p] = 1.0
    sel = np.zeros((2, 128), F32)
    sel[0, 0:64] = 1.0
    sel[1, 64:128] = 1.0

    common = {
        "ctxin": np.ascontiguousarray(ctx_fm.reshape(2, CH, 128, NT)),
        "gnw": gnw, "gnb": gnb, "gsum": gsum, "e8": e8, "sel": sel,
        "bias": bias.astype(BF16),
    }
    for name in ("wq", "wk", "wv", "wo", "cawq", "cawk", "cawv", "cawo"):
        common[name] = np.ascontiguousarray(
            w[name].astype(BF16).reshape(CH, 128, 512))
    for name, src in (("diag", "diag"), ("cadiag", "cadiag")):
        d4 = np.zeros((CH, 128, 128), F32)
        for c in range(CH):
            np.fill_diagonal(d4[c], w[src][c * 128:(c + 1) * 128])
        common[name] = d4.astype(BF16)

    in_maps = []
    for cid in range(N_CORES):
        b, j = cid // 4, cid % 4
        fA = 2 * j
        prev = max(fA - 1, 0)
        xloc = np.stack([x[b * T + prev], x[b * T + fA], x[b * T + fA + 1]])
        m = dict(common)
        m["xin"] = np.ascontiguousarray(xloc.reshape(3, CH, 128, HW))
        in_maps.append(m)
    return in_maps


def assemble(results, HW):
    out = np.empty((B * T, C, HW), F32)
    for cid in range(N_CORES):
        b, j = cid // 4, cid % 4
        o = results[cid]["out"]
        out[b * T + 2 * j] = o[0].reshape(C, HW)
        out[b * T + 2 * j + 1] = o[1].reshape(C, HW)
    H = int(round(np.sqrt(HW)))
    return out.reshape(B * T, C, H, H)


_CACHE = {}


def _get_module(HW=1024):
    if HW not in _CACHE:
        _CACHE[HW] = build_module(HW=HW)
    return _CACHE[HW]


def kernel(**inputs):
    from concourse.bass_utils import run_bass_kernel_spmd

    inp = {k: np.asarray(v, F32) for k, v in inputs.items()}
    HW = inp['x'].shape[2] * inp['x'].shape[3]
    nc = _get_module(HW)
    in_maps = make_in_maps(inp, HW)
    res = run_bass_kernel_spmd(nc, in_maps, core_ids=list(range(N_CORES)))
    return assemble(res.results, HW)
